# revision 26
# baseline (speedup 1.0000x reference)
"""Causal self-attention (B=4, T=2048, C=1024, H=16) on 8 trn2 cores.

Sharding: batch (4-way) x head-group (2-way).  Core i handles batch i//2 and
heads [8*(i%2), 8*(i%2)+8).  Each core computes qkv projection for its head
slice, causal attention, and a partial out-projection (contraction over its
512 att columns).  Host sums the two partials per batch.

All device matmul operands are bf16 (PSUM accumulation stays fp32).  Host
pre-transposes and pre-converts so the device never transposes or converts:
  - xT       (C, T)      : x[b].T
  - wqk      (8,128,8*128): per m-tile of [wq_g; wk_g].T, k-tiles along free
  - wv       (C, 512)    : wv_g.T
  - wo       (512, C)    : w_out.T row-slice for this head group
  - masks    (128, 1280) : packed binary causal masks for the (narrowed)
                           diagonal tile pairs: [512|384|256|128] variants
  - vinit    (128, 772)  : v_sb ones/zeros init pattern
  - ones_in  (128, 128)  : all-ones lhsT for the PE sums-broadcast matmul
Layouts on chip:
  - QT/KT  [128, 4, T]   rows = head-major (hl*64+d), T on free dim
  - V      [128, 16, 772]: per t-tile, per head pair [V_e|1] + [1|0*63|V_o]
  - attT   [128, 4, T]   rows = c_local = hl*64+d  (lhsT for out-proj)

v2 structure -- fully software-pipelined phases.  The softmax exp on
ScalarE (~158us total) is the hard serial floor of the attention math, so
the kernel is organized to keep ScalarE saturated while the PE consumes
qkv-projection and out-projection work in the gaps:
  - qkv runs per t-QUARTER (512 cols): quarter 0 up front, quarter j+1
    emitted as filler units INSIDE attention chunk j.  Each unit is one
    (wave, m-tile) = 8 accumulating matmuls + one psum->sbuf cast, with
    all inputs long since DMA'd -- it can never stall the PE.
  - attention chunks processed j = 0,1,2,3; chunk j's out-projection is
    split into 8 sub-blocks (per t-tile o-half: 4 matmuls + bf16 cast +
    DMA) consumed as filler inside chunk j+1 (only after the previous
    chunk's deferred finalize has run -- they read its attT columns).
  - score matmuls for the two heads of a pair are emitted interleaved;
    s0 uses array rows 0-63, s1 rows 64-127 (tile_position auto-derived
    from base_partition), so the PE runs them CONCURRENTLY in disjoint
    row groups, halving effective score-matmul time.
  - keep-warm dummy matmuls only when no real filler is available (a PE
    idle window >~3.4us re-throttles the HAM clock gate to 1.2 GHz).
  - y is stored as bf16 (host converts + sums partials in f32).
PSUM: st 2x[128,1024] (4 banks) + av 2x[128,512] (2) + p1 2x[128,512]
(2, shared by qkv units / out-proj sub-blocks / sums-broadcast) = 8.
"""

from contextlib import ExitStack

import ml_dtypes
import numpy as np

import concourse.bass as bass
import concourse.mybir as mybir
import concourse.tile as tile
from concourse import bacc, bass_utils

B, T, C, H, HD = 4, 2048, 1024, 16, 64
HG = 2  # head groups (tensor-parallel dim)
HPG = H // HG  # 8 heads per group
OG = HPG * HD  # 512: local width of q/k/v slice
KT_C = C // 128  # 8 contraction tiles for the projections
NT = T // 128  # 16 t-tiles
NQ = T // 512  # 4 tq chunks
PAIR_W = 65 + 128  # v_sb cols per head pair: [V_e|1] + [0*63|1|V_o]
DN = 512  # keep-warm filler matmul width (~213ns at full clock)

f32 = mybir.dt.float32
bf16 = mybir.dt.bfloat16

TRACE = False  # test.py flips this for profiling runs
DEBUG = False
LAST_RUN = {}

_NC_CACHE = []


def _mm(nc, out, lhsT, rhs, **kw):
    nc.tensor.matmul(out, lhsT, rhs, **kw)


def _build_nc():
    nc = bacc.Bacc(trn_type="TRN2", target_bir_lowering=False, debug=False)
    xT = nc.dram_tensor("xT", [C, T], bf16, kind="ExternalInput").ap()
    wqk = nc.dram_tensor("wqk", [8, 128, 1024], bf16, kind="ExternalInput").ap()
    wv = nc.dram_tensor("wv", [C, OG], bf16, kind="ExternalInput").ap()
    wo = nc.dram_tensor("wo", [OG, C], bf16, kind="ExternalInput").ap()
    masks = nc.dram_tensor("masks", [128, 1280], bf16, kind="ExternalInput").ap()
    vinit = nc.dram_tensor("vinit", [128, 4 * PAIR_W], bf16, kind="ExternalInput").ap()
    ones_in = nc.dram_tensor("ones_in", [128, 128], bf16, kind="ExternalInput").ap()
    y = nc.dram_tensor("y", [T, C], bf16, kind="ExternalOutput").ap()

    with tile.TileContext(nc) as tc:
        _body(tc, nc, xT, wqk, wv, wo, masks, vinit, ones_in, y)
    nc.compile()
    return nc


def _body(tc, nc, xT, wqk, wv, wo, masks, vinit, ones_in, y):
    exp_f = mybir.ActivationFunctionType.Exp

    with (
        tc.tile_pool(name="persist", bufs=1) as persist,
        tc.tile_pool(name="mask_p", bufs=1) as mask_p,
        tc.tile_pool(name="ones_p", bufs=1) as ones_p,
        tc.tile_pool(name="wo_p", bufs=1) as wo_p,
        tc.tile_pool(name="wv_p", bufs=1) as wv_p,
        tc.tile_pool(name="xh_p", bufs=4) as xh_p,
        tc.tile_pool(name="wqk_p", bufs=1) as wqk_p,
        tc.tile_pool(name="pt_p", bufs=4) as pt_p,
        tc.tile_pool(name="recip_p", bufs=2) as recip_p,
        tc.tile_pool(name="bcast_p", bufs=2) as bcast_p,
        tc.tile_pool(name="yo_p", bufs=2) as yo_p,
        tc.tile_pool(name="st_ps", bufs=2, space="PSUM") as st_ps,
        tc.tile_pool(name="av_ps", bufs=2, space="PSUM") as av_ps,
        tc.tile_pool(name="p1_ps", bufs=2, space="PSUM") as p1_ps,
    ):
        qt = persist.tile([128, 4, T], bf16)
        kt = persist.tile([128, 4, T], bf16)
        v_sb = persist.tile([128, NT, 4 * PAIR_W], bf16)
        attT = persist.tile([128, 4, T], bf16)
        mk = mask_p.tile([128, 1280], bf16)
        ones_sb = ones_p.tile([128, 128], bf16)
        wo_sb = wo_p.tile([128, 4, C], bf16)
        wv_sb = wv_p.tile([128, KT_C, OG], bf16)

        # ---- input DMAs (ordered so the first qkv matmul's deps land first)
        wts = {}
        xh = [[None] * KT_C for _ in range(4)]  # [quarter][k]

        def load_xh(qq, k):
            t = xh_p.tile([128, 512], bf16, tag=f"xh{k}", name=f"xh_{qq}_{k}")
            xh[qq][k] = t
            nc.sync.dma_start(
                t[:], xT[k * 128 : (k + 1) * 128, qq * 512 : (qq + 1) * 512]
            )

        def load_wt(m):
            wt = wqk_p.tile([128, 1024], bf16, tag=f"wqk{m}", name=f"wt{m}")
            wts[m] = wt
            nc.sync.dma_start(wt[:], wqk[m, :, :])

        for m in range(2):
            load_wt(m)
        for k in range(KT_C):
            load_xh(0, k)
        for m in range(2, 8):
            load_wt(m)
        for k in range(KT_C):
            nc.sync.dma_start(wv_sb[:, k, :], wv[k * 128 : (k + 1) * 128, :])
        for tt in range(NT):
            nc.sync.dma_start(v_sb[:, tt, :], vinit[:])
        nc.sync.dma_start(mk[:], masks[:])
        nc.sync.dma_start(ones_sb[:], ones_in[:])
        for k in range(4):
            nc.sync.dma_start(wo_sb[:, k, :], wo[k * 128 : (k + 1) * 128, :])
        for qq in range(1, 4):
            for k in range(KT_C):
                load_xh(qq, k)

        # ---- qkv filler units: one (wave, m-tile) or (v, t-tile) each
        def qkv_units(qq):
            tq0 = qq * 512
            units = []
            for m0, dst in ((0, qt), (4, kt)):
                for mi in range(4):
                    def u(mi=mi, m0=m0, dst=dst, qq=qq, tq0=tq0):
                        ps = p1_ps.tile(
                            [128, 512], f32, tag="p1", name=f"p1_{qq}_{m0}_{mi}"
                        )
                        wt = wts[m0 + mi]
                        for k in range(KT_C):
                            _mm(
                                nc,
                                ps[:],
                                wt[:, k * 128 : (k + 1) * 128],
                                xh[qq][k][:],
                                start=(k == 0),
                                stop=(k == KT_C - 1),
                            )
                        nc.vector.tensor_copy(dst[:, mi, tq0 : tq0 + 512], ps[:])
                    units.append(u)
            for tl2 in range(4):
                def u(tl2=tl2, qq=qq):
                    tt = qq * 4 + tl2
                    ps = p1_ps.tile(
                        [128, 512], f32, tag="p1", name=f"p1v_{qq}_{tl2}"
                    )
                    for k in range(KT_C):
                        _mm(
                            nc,
                            ps[:],
                            xh[qq][k][:, tl2 * 128 : (tl2 + 1) * 128],
                            wv_sb[:, k, :],
                            start=(k == 0),
                            stop=(k == KT_C - 1),
                        )
                    src_e = ps[:].rearrange("p (h d) -> p h d", d=64)[:, 0::2, :]
                    src_o = ps[:].rearrange("p (h d) -> p h d", d=64)[:, 1::2, :]
                    dstv = v_sb[:, tt, :].rearrange("p (q w) -> p q w", w=PAIR_W)
                    nc.vector.tensor_copy(dstv[:, :, 0:64], src_e)
                    nc.vector.tensor_copy(dstv[:, :, 129:193], src_o)
                units.append(u)
            return units

        # ---- out-projection sub-blocks (filler units for the next chunk)
        def outproj_sub(tt, o):
            yps = p1_ps.tile([128, 512], f32, tag="p1", name=f"yp_{tt}_{o}")
            for k in range(4):
                _mm(
                    nc,
                    yps[:],
                    attT[:, k, tt * 128 : (tt + 1) * 128],
                    wo_sb[:, k, o * 512 : (o + 1) * 512],
                    start=(k == 0),
                    stop=(k == 3),
                )
            yo = yo_p.tile([128, 512], bf16, tag="yo", name=f"yo_{tt}_{o}")
            nc.vector.tensor_copy(yo[:], yps[:])
            nc.sync.dma_start(
                y[tt * 128 : (tt + 1) * 128, o * 512 : (o + 1) * 512], yo[:]
            )

        def outproj_units(j):
            return [
                (lambda tt, oo: (lambda: outproj_sub(tt, oo)))(4 * j + tl, o)
                for tl in range(4)
                for o in range(2)
            ]

        # ---- attention ----
        def head_ctx(hl):
            p0 = (hl % 2) * 64
            mt = hl // 2
            qrow = slice(p0, p0 + 64)
            vb0 = (hl // 2) * PAIR_W
            if hl % 2 == 0:
                vsl = (vb0, vb0 + 65)  # [V|1] -> rows 0..64
                srow, arow = 64, slice(0, 64)
            else:
                vsl = (vb0 + 65, vb0 + 193)  # [1|0*63|V] -> row 0 sums
                srow, arow = 0, slice(64, 128)
            return p0, mt, qrow, vsl, srow, arow

        pend_fin = [None]  # deferred finalize of the previous head pair
        pend_qkv = []  # next quarter's qkv units (safe any time)
        pend_op = []  # prev chunk's out-proj units (safe after pend_fin)
        op_safe = [False]

        def do_chunk(j):
            ntk = 4 * j + 4
            ng = ntk // 2
            tq = slice(j * 512, (j + 1) * 512)
            for ha in range(0, HPG, 2):
                ctxs = [head_ctx(ha), head_ctx(ha + 1)]
                pts = {0: [None] * ng, 1: [None] * ng}
                avs = [
                    av_ps.tile([128, 512], f32, tag="av", name=f"av{s}_{ha}_{j}")
                    for s in (0, 1)
                ]

                def dummy(s):
                    # keep-warm filler matmul into the dead rows of the av
                    # accumulator (the complement of each head's rows is
                    # never read).  A stalled PE re-throttles the HAM clock
                    # gate to 1.2 GHz, doubling every matmul.
                    rows = slice(96, 128) if s == 0 else slice(32, 64)
                    _mm(
                        nc,
                        avs[s][rows, 0:DN],
                        mk[:, 64 : 64 + (rows.stop - rows.start)],
                        mk[:, 0:DN],
                        start=True,
                        stop=True,
                        skip_group_check=True,
                        tile_position=(0, rows.start),
                    )

                def filler(s):
                    if pend_qkv:
                        pend_qkv.pop(0)()
                    elif op_safe[0] and pend_op:
                        pend_op.pop(0)()
                    else:
                        dummy(s)

                def emit_group(g):
                    geom = []  # per u: (tq_off, width, pt_col)
                    pcol = 0
                    for u in range(2):
                        tk = 2 * g + u
                        v = tk - 4 * j
                        off = 128 * v if v > 0 else 0
                        w = 512 - off
                        geom.append((off, w, pcol))
                        pcol = 512 if u == 0 and w == 512 else pcol + w
                    dg = 2 * g - 4 * j
                    pss = [
                        st_ps.tile(
                            [128, 1024], f32, tag="st", name=f"st{s}_{ha}_{j}_{g}"
                        )
                        for s in (0, 1)
                    ]
                    # two heads' score matmuls interleaved: s0 in array rows
                    # 0-63, s1 in rows 64-127 -> concurrent row groups.
                    for u in range(2):
                        off, w, pc = geom[u]
                        tk = 2 * g + u
                        for s in (0, 1):
                            _, mt, qrow, _, _, _ = ctxs[s]
                            _mm(
                                nc,
                                pss[s][:, pc : pc + w],
                                kt[qrow, mt, tk * 128 : (tk + 1) * 128],
                                qt[qrow, mt, j * 512 + off : (j + 1) * 512],
                                start=True,
                                stop=True,
                            )
                    tot = geom[1][2] + geom[1][1]
                    for s in (0, 1):
                        pt = pt_p.tile([128, 1024], bf16, tag=f"pt{s}")
                        nc.scalar.activation(
                            pt[:, 0:tot], pss[s][:, 0:tot], exp_f, scale=0.125
                        )
                        if dg == 0:  # pair (4j, 4j+1): widths 512|384
                            nc.vector.tensor_mul(
                                pt[:, 0:896], pt[:, 0:896], mk[:, 0:896]
                            )
                        elif dg == 2:  # pair (4j+2, 4j+3): widths 256|128
                            nc.vector.tensor_mul(
                                pt[:, 0:384], pt[:, 0:384], mk[:, 896:1280]
                            )
                        pts[s][g] = (pt, geom)

                emit_group(0)
                if pend_fin[0] is not None:
                    # two finalize halves bracket a filler so the PE has work
                    # while p1_ps rotates through the s0 reciprocal.  qkv
                    # fillers are safe here; out-proj fillers are NOT until
                    # f1 has written its attT columns (op_safe gating).
                    f0, f1 = pend_fin[0]
                    f0()
                    filler(0)
                    f1()
                    pend_fin[0] = None
                    op_safe[0] = True
                else:
                    filler(0)
                if ha == 0:
                    # chunk entry is a PE-density dip (boundary drains are
                    # empty, first scores wait on the st rotation): the
                    # 17us HAM cold window at the c2->c3 boundary starts
                    # here.  Two extra fillers carry the PE through it --
                    # at c3 entry these pop the just-stocked out-proj units.
                    filler(0)
                    filler(1)
                if ng > 1:
                    emit_group(1)

                for g in range(ng):
                    if g + 2 < ng:
                        filler(0)
                        emit_group(g + 2)
                        filler(1)
                    for u in range(2):
                        for s in (0, 1):
                            _, _, _, vsl, _, _ = ctxs[s]
                            pt, geom = pts[s][g]
                            off, w, pc = geom[u]
                            tk = 2 * g + u
                            _mm(
                                nc,
                                avs[s][0 : vsl[1] - vsl[0], off : off + w],
                                v_sb[:, tk, vsl[0] : vsl[1]],
                                pt[:, pc : pc + w],
                                start=(tk == 0),
                                stop=(tk == ntk - 1),
                            )

                filler(1)
                # sums copies go out immediately (DVE, overlaps next pair's
                # score matmuls); the rest is deferred via pend_fin.
                sums = []
                for s in (0, 1):
                    _, _, _, _, srow, _ = ctxs[s]
                    av = avs[s]
                    sums_sb = recip_p.tile([128, 512], bf16, tag=f"rc{s}")
                    nc.vector.tensor_copy(
                        sums_sb[srow : srow + 1, :], av[srow : srow + 1, :]
                    )
                    sums.append(sums_sb)

                def fin_half(s, ctxs=ctxs, avs=avs, sums=sums, ha=ha, j=j, tq=tq):
                    _, mt, _, _, srow, arow = ctxs[s]
                    bps = p1_ps.tile(
                        [128, 512], f32, tag="p1", name=f"bps_{ha}_{j}_{s}"
                    )
                    _mm(
                        nc,
                        bps[:],
                        ones_sb[srow : srow + 1, :],
                        sums[s][srow : srow + 1, :],
                        start=True,
                        stop=True,
                    )
                    bc = bcast_p.tile([128, 512], f32, tag=f"bc{s}")
                    nc.vector.reciprocal_approx_fast(bc[:], bps[:])
                    nc.vector.tensor_mul(
                        attT[arow, mt, tq], avs[s][arow, :], bc[arow, :]
                    )

                pend_fin[0] = (lambda: fin_half(0), lambda: fin_half(1))

        # ---- main schedule ----
        for u in qkv_units(0):
            u()
        for j in range(NQ):
            pend_qkv[:] = qkv_units(j + 1) if j < 3 else []
            op_safe[0] = pend_fin[0] is None
            do_chunk(j)
            # drain leftovers: next chunk's scores depend on the qkv casts;
            # out-proj leftovers are safe (this chunk consumed pend_fin).
            for u in pend_qkv:
                u()
            pend_qkv[:] = []
            for u in pend_op:
                u()
            pend_op[:] = outproj_units(j) if j < 3 else []
        f0, f1 = pend_fin[0]
        f0()
        f1()
        pend_fin[0] = None
        # last chunk's out-projection: the score pipeline is done, so the
        # st pool (2x [128,1024]) is free -- whole-t-tile blocks with the
        # o-halves alternating across the tile's two banks (same-bank
        # back-to-back accumulation runs at isolated-MM latency; alternating
        # banks pipelines at ~220ns), 2-buf pipelined so tile N+1's matmuls
        # overlap tile N's cast + DMA.
        for tl in range(4):
            tt = 12 + tl
            yps = st_ps.tile([128, 1024], f32, tag="st", name=f"yfin_{tt}")
            for k in range(4):
                for o in range(2):
                    _mm(
                        nc,
                        yps[:, o * 512 : (o + 1) * 512],
                        attT[:, k, tt * 128 : (tt + 1) * 128],
                        wo_sb[:, k, o * 512 : (o + 1) * 512],
                        start=(k == 0),
                        stop=(k == 3),
                    )
            yof = yo_p.tile([128, 1024], bf16, tag="yof", name=f"yof_{tt}")
            nc.vector.tensor_copy(yof[:], yps[:])
            nc.sync.dma_start(y[tt * 128 : (tt + 1) * 128, :], yof[:])


def _host_prep(x, w_qkv, w_out):
    def to_bf(a):
        return np.ascontiguousarray(a, dtype=np.float32).astype(ml_dtypes.bfloat16)

    xT_all = to_bf(x.transpose(0, 2, 1))
    tk_l = np.arange(128)[:, None]
    m0 = (np.arange(512)[None, :] >= tk_l).astype(np.float32)
    masks = to_bf(
        np.concatenate([m0, m0[:, :384], m0[:, :256], m0[:, :128]], axis=1)
    )

    per_group = []
    for g in range(HG):
        wq = w_qkv[g * OG : (g + 1) * OG]
        wk = w_qkv[C + g * OG : C + (g + 1) * OG]
        wvg = w_qkv[2 * C + g * OG : 2 * C + (g + 1) * OG]
        wqkT = np.concatenate([wq, wk], axis=0).T  # (C, 1024)
        wqk_r = to_bf(
            wqkT.reshape(8, 128, 8, 128).transpose(2, 1, 0, 3).reshape(8, 128, 1024)
        )
        wv_t = to_bf(wvg.T)  # (C, 512)
        wo_t = to_bf(w_out.T[g * OG : (g + 1) * OG])  # (512, C)
        per_group.append((wqk_r, wv_t, wo_t))
    vinit = np.zeros((128, 4 * PAIR_W), np.float32)
    for pr in range(4):
        vinit[:, pr * PAIR_W + 64] = 1.0  # even-head ones col
        vinit[:, pr * PAIR_W + 65] = 1.0  # odd-head ones col (block col 0)
    ones_in = to_bf(np.ones((128, 128), np.float32))
    return xT_all, masks, to_bf(vinit), ones_in, per_group


def kernel(x, w_qkv, w_out):
    x = np.asarray(x)
    w_qkv = np.asarray(w_qkv)
    w_out = np.asarray(w_out)
    xT_all, masks, vinit, ones_in, per_group = _host_prep(x, w_qkv, w_out)

    if not _NC_CACHE:
        _NC_CACHE.append(_build_nc())
    nc = _NC_CACHE[0]

    in_maps = []
    for core in range(8):
        b, g = core // 2, core % 2
        wqk_r, wv_t, wo_t = per_group[g]
        in_maps.append(
            {"xT": xT_all[b], "wqk": wqk_r, "wv": wv_t, "wo": wo_t, "masks": masks,
             "vinit": vinit, "ones_in": ones_in}
        )

    res = bass_utils.run_bass_kernel_spmd(
        nc, in_maps, core_ids=list(range(8)), trace=TRACE
    )
    LAST_RUN["res"] = res

    y = np.empty((B, T, C), np.float32)
    for b in range(B):
        y[b] = np.asarray(res.results[2 * b]["y"], np.float32) + np.asarray(
            res.results[2 * b + 1]["y"], np.float32
        )
    return y


# revision 31
# speedup vs baseline: 1.2792x; 1.2792x over previous
"""Causal self-attention (B=4, T=2048, C=1024, H=16) on 8 trn2 cores.

Sharding: batch (4-way) x head-group (2-way).  Core i handles batch i//2 and
heads [8*(i%2), 8*(i%2)+8).  Each core computes qkv projection for its head
slice, causal attention, and a partial out-projection (contraction over its
512 att columns).  Host sums the two partials per batch.

All device matmul operands are bf16 (PSUM accumulation stays fp32).  Host
pre-transposes and pre-converts so the device never transposes or converts:
  - xT       (C, T)      : x[b].T
  - wqk      (8,128,8*128): per m-tile of [wq_g; wk_g].T, k-tiles along free
  - wv       (C, 512)    : wv_g.T
  - wo       (512, C)    : w_out.T row-slice for this head group
  - masks    (128, 1280) : packed binary causal masks for the (narrowed)
                           diagonal tile pairs: [512|384|256|128] variants
  - vinit    (128, 772)  : v_sb ones/zeros init pattern
  - ones_in  (128, 128)  : all-ones lhsT for the PE sums-broadcast matmul
Layouts on chip:
  - QT/KT  [128, 4, T]   rows = head-major (hl*64+d), T on free dim
  - V      [128, 16, 772]: per t-tile, per head pair [V_e|1] + [1|0*63|V_o]
  - attT   [128, 4, T]   rows = c_local = hl*64+d  (lhsT for out-proj)

v2 structure -- fully software-pipelined phases.  The softmax exp on
ScalarE (~158us total) is the hard serial floor of the attention math, so
the kernel is organized to keep ScalarE saturated while the PE consumes
qkv-projection and out-projection work in the gaps:
  - qkv runs per t-QUARTER (512 cols): quarter 0 up front, quarter j+1
    emitted as filler units INSIDE attention chunk j.  Each unit is one
    (wave, m-tile) = 8 accumulating matmuls + one psum->sbuf cast, with
    all inputs long since DMA'd -- it can never stall the PE.
  - attention chunks processed j = 0,1,2,3; chunk j's out-projection is
    split into 8 sub-blocks (per t-tile o-half: 4 matmuls + bf16 cast +
    DMA) consumed as filler inside chunk j+1 (only after the previous
    chunk's deferred finalize has run -- they read its attT columns).
  - score matmuls for the two heads of a pair are emitted interleaved;
    s0 uses array rows 0-63, s1 rows 64-127 (tile_position auto-derived
    from base_partition), so the PE runs them CONCURRENTLY in disjoint
    row groups, halving effective score-matmul time.
  - keep-warm dummy matmuls only when no real filler is available (a PE
    idle window >~3.4us re-throttles the HAM clock gate to 1.2 GHz).
  - y is stored as bf16 (host converts + sums partials in f32).
PSUM: st 2x[128,1024] (4 banks) + av 2x[128,512] (2) + p1 2x[128,512]
(2, shared by qkv units / out-proj sub-blocks / sums-broadcast) = 8.
"""

from contextlib import ExitStack

import ml_dtypes
import numpy as np

import concourse.bass as bass
import concourse.mybir as mybir
import concourse.tile as tile
from concourse import bacc, bass_utils

B, T, C, H, HD = 4, 2048, 1024, 16, 64
HG = 2  # head groups (tensor-parallel dim)
HPG = H // HG  # 8 heads per group
OG = HPG * HD  # 512: local width of q/k/v slice
KT_C = C // 128  # 8 contraction tiles for the projections
NT = T // 128  # 16 t-tiles
NQ = T // 512  # 4 tq chunks
PAIR_W = 65 + 128  # v_sb cols per head pair: [V_e|1] + [0*63|1|V_o]
DN = 512  # keep-warm filler matmul width (~213ns at full clock)

f32 = mybir.dt.float32
bf16 = mybir.dt.bfloat16

TRACE = False  # test.py flips this for profiling runs
DEBUG = False
LAST_RUN = {}

_NC_CACHE = []


def _mm(nc, out, lhsT, rhs, **kw):
    nc.tensor.matmul(out, lhsT, rhs, **kw)


def _build_nc():
    nc = bacc.Bacc(trn_type="TRN2", target_bir_lowering=False, debug=False)
    xT = nc.dram_tensor("xT", [C, T], bf16, kind="ExternalInput").ap()
    wqk = nc.dram_tensor("wqk", [8, 128, 1024], bf16, kind="ExternalInput").ap()
    wv = nc.dram_tensor("wv", [C, OG], bf16, kind="ExternalInput").ap()
    wo = nc.dram_tensor("wo", [OG, C], bf16, kind="ExternalInput").ap()
    masks = nc.dram_tensor("masks", [128, 1280], bf16, kind="ExternalInput").ap()
    vinit = nc.dram_tensor("vinit", [128, 4 * PAIR_W], bf16, kind="ExternalInput").ap()
    ones_in = nc.dram_tensor("ones_in", [128, 128], bf16, kind="ExternalInput").ap()
    y = nc.dram_tensor("y", [T, C], bf16, kind="ExternalOutput").ap()

    with tile.TileContext(nc) as tc:
        _body(tc, nc, xT, wqk, wv, wo, masks, vinit, ones_in, y)
    nc.compile()
    return nc


def _body(tc, nc, xT, wqk, wv, wo, masks, vinit, ones_in, y):
    exp_f = mybir.ActivationFunctionType.Exp

    with (
        tc.tile_pool(name="persist", bufs=1) as persist,
        tc.tile_pool(name="mask_p", bufs=1) as mask_p,
        tc.tile_pool(name="ones_p", bufs=1) as ones_p,
        tc.tile_pool(name="wo_p", bufs=1) as wo_p,
        tc.tile_pool(name="wv_p", bufs=1) as wv_p,
        tc.tile_pool(name="xh_p", bufs=4) as xh_p,
        tc.tile_pool(name="wqk_p", bufs=1) as wqk_p,
        tc.tile_pool(name="pt_p", bufs=4) as pt_p,
        tc.tile_pool(name="recip_p", bufs=2) as recip_p,
        tc.tile_pool(name="bcast_p", bufs=2) as bcast_p,
        tc.tile_pool(name="yo_p", bufs=2) as yo_p,
        tc.tile_pool(name="st_ps", bufs=2, space="PSUM") as st_ps,
        tc.tile_pool(name="av_ps", bufs=2, space="PSUM") as av_ps,
        tc.tile_pool(name="p1_ps", bufs=2, space="PSUM") as p1_ps,
    ):
        qt = persist.tile([128, 4, T], bf16)
        kt = persist.tile([128, 4, T], bf16)
        v_sb = persist.tile([128, NT, 4 * PAIR_W], bf16)
        attT = persist.tile([128, 4, T], bf16)
        mk = mask_p.tile([128, 1280], bf16)
        ones_sb = ones_p.tile([128, 128], bf16)
        wo_sb = wo_p.tile([128, 4, C], bf16)
        wv_sb = wv_p.tile([128, KT_C, OG], bf16)

        # ---- input DMAs (ordered so the first qkv matmul's deps land first)
        wts = {}
        xh = [[None] * KT_C for _ in range(4)]  # [quarter][k]

        def load_xh(qq, k):
            t = xh_p.tile([128, 512], bf16, tag=f"xh{k}", name=f"xh_{qq}_{k}")
            xh[qq][k] = t
            nc.sync.dma_start(
                t[:], xT[k * 128 : (k + 1) * 128, qq * 512 : (qq + 1) * 512]
            )

        def load_wt(m):
            wt = wqk_p.tile([128, 1024], bf16, tag=f"wqk{m}", name=f"wt{m}")
            wts[m] = wt
            nc.sync.dma_start(wt[:], wqk[m, :, :])

        for m in range(2):
            load_wt(m)
        for k in range(KT_C):
            load_xh(0, k)
        for m in range(2, 8):
            load_wt(m)
        for k in range(KT_C):
            nc.sync.dma_start(wv_sb[:, k, :], wv[k * 128 : (k + 1) * 128, :])
        for tt in range(NT):
            nc.sync.dma_start(v_sb[:, tt, :], vinit[:])
        nc.sync.dma_start(mk[:], masks[:])
        nc.sync.dma_start(ones_sb[:], ones_in[:])
        for k in range(4):
            nc.sync.dma_start(wo_sb[:, k, :], wo[k * 128 : (k + 1) * 128, :])
        for qq in range(1, 4):
            for k in range(KT_C):
                load_xh(qq, k)

        # ---- qkv filler units: one (wave, m-tile) or (v, t-tile) each
        def qkv_units(qq):
            tq0 = qq * 512
            units = []
            for m0, dst in ((0, qt), (4, kt)):
                for mi in range(4):
                    def u(mi=mi, m0=m0, dst=dst, qq=qq, tq0=tq0):
                        ps = p1_ps.tile(
                            [128, 512], f32, tag="p1", name=f"p1_{qq}_{m0}_{mi}"
                        )
                        wt = wts[m0 + mi]
                        for k in range(KT_C):
                            _mm(
                                nc,
                                ps[:],
                                wt[:, k * 128 : (k + 1) * 128],
                                xh[qq][k][:],
                                start=(k == 0),
                                stop=(k == KT_C - 1),
                            )
                        nc.vector.tensor_copy(dst[:, mi, tq0 : tq0 + 512], ps[:])
                    units.append(u)
            for tl2 in range(4):
                def u(tl2=tl2, qq=qq):
                    tt = qq * 4 + tl2
                    ps = p1_ps.tile(
                        [128, 512], f32, tag="p1", name=f"p1v_{qq}_{tl2}"
                    )
                    for k in range(KT_C):
                        _mm(
                            nc,
                            ps[:],
                            xh[qq][k][:, tl2 * 128 : (tl2 + 1) * 128],
                            wv_sb[:, k, :],
                            start=(k == 0),
                            stop=(k == KT_C - 1),
                        )
                    src_e = ps[:].rearrange("p (h d) -> p h d", d=64)[:, 0::2, :]
                    src_o = ps[:].rearrange("p (h d) -> p h d", d=64)[:, 1::2, :]
                    dstv = v_sb[:, tt, :].rearrange("p (q w) -> p q w", w=PAIR_W)
                    nc.vector.tensor_copy(dstv[:, :, 0:64], src_e)
                    nc.vector.tensor_copy(dstv[:, :, 129:193], src_o)
                units.append(u)
            return units

        # ---- out-projection sub-blocks (filler units for the next chunk)
        def outproj_sub(tt, o):
            yps = p1_ps.tile([128, 512], f32, tag="p1", name=f"yp_{tt}_{o}")
            for k in range(4):
                _mm(
                    nc,
                    yps[:],
                    attT[:, k, tt * 128 : (tt + 1) * 128],
                    wo_sb[:, k, o * 512 : (o + 1) * 512],
                    start=(k == 0),
                    stop=(k == 3),
                )
            yo = yo_p.tile([128, 512], bf16, tag="yo", name=f"yo_{tt}_{o}")
            nc.vector.tensor_copy(yo[:], yps[:])
            nc.sync.dma_start(
                y[tt * 128 : (tt + 1) * 128, o * 512 : (o + 1) * 512], yo[:]
            )

        def outproj_units(j):
            return [
                (lambda tt, oo: (lambda: outproj_sub(tt, oo)))(4 * j + tl, o)
                for tl in range(4)
                for o in range(2)
            ]

        # ---- attention ----
        def head_ctx(hl):
            p0 = (hl % 2) * 64
            mt = hl // 2
            qrow = slice(p0, p0 + 64)
            vb0 = (hl // 2) * PAIR_W
            if hl % 2 == 0:
                vsl = (vb0, vb0 + 65)  # [V|1] -> rows 0..64
                srow, arow = 64, slice(0, 64)
            else:
                vsl = (vb0 + 65, vb0 + 193)  # [1|0*63|V] -> row 0 sums
                srow, arow = 0, slice(64, 128)
            return p0, mt, qrow, vsl, srow, arow

        pend_fin = [None]  # deferred finalize of the previous head pair
        pend_qkv = []  # next quarter's qkv units (safe any time)
        pend_op = []  # prev chunk's out-proj units (safe after pend_fin)
        op_safe = [False]

        def do_chunk(j):
            ntk = 4 * j + 4
            ng = ntk // 2
            tq = slice(j * 512, (j + 1) * 512)
            carry = [None]  # (ha, ctxs, pts) of a pre-emitted next pair
            for ha in range(0, HPG, 2):
                if carry[0] is not None and carry[0][0] == ha:
                    _, ctxs, pts = carry[0]
                    carry[0] = None
                    eg0_done = True
                else:
                    ctxs = [head_ctx(ha), head_ctx(ha + 1)]
                    pts = {0: [None] * ng, 1: [None] * ng}
                    eg0_done = False
                avs = [
                    av_ps.tile([128, 512], f32, tag="av", name=f"av{s}_{ha}_{j}")
                    for s in (0, 1)
                ]

                def dummy(s):
                    # keep-warm filler matmul into the dead rows of the av
                    # accumulator (the complement of each head's rows is
                    # never read).  A stalled PE re-throttles the HAM clock
                    # gate to 1.2 GHz, doubling every matmul.
                    rows = slice(96, 128) if s == 0 else slice(32, 64)
                    _mm(
                        nc,
                        avs[s][rows, 0:DN],
                        mk[:, 64 : 64 + (rows.stop - rows.start)],
                        mk[:, 0:DN],
                        start=True,
                        stop=True,
                        skip_group_check=True,
                        tile_position=(0, rows.start),
                    )

                def filler(s):
                    if pend_qkv:
                        pend_qkv.pop(0)()
                    elif op_safe[0] and pend_op:
                        pend_op.pop(0)()
                    else:
                        dummy(s)

                def emit_group(g, ctxs=ctxs, pts=pts, ha=ha):
                    geom = []  # per u: (tq_off, width, pt_col)
                    pcol = 0
                    for u in range(2):
                        tk = 2 * g + u
                        v = tk - 4 * j
                        off = 128 * v if v > 0 else 0
                        w = 512 - off
                        geom.append((off, w, pcol))
                        pcol = 512 if u == 0 and w == 512 else pcol + w
                    dg = 2 * g - 4 * j
                    pss = [
                        st_ps.tile(
                            [128, 1024], f32, tag="st", name=f"st{s}_{ha}_{j}_{g}"
                        )
                        for s in (0, 1)
                    ]
                    # two heads' score matmuls interleaved: s0 in array rows
                    # 0-63, s1 in rows 64-127 -> concurrent row groups.
                    for u in range(2):
                        off, w, pc = geom[u]
                        tk = 2 * g + u
                        for s in (0, 1):
                            _, mt, qrow, _, _, _ = ctxs[s]
                            _mm(
                                nc,
                                pss[s][:, pc : pc + w],
                                kt[qrow, mt, tk * 128 : (tk + 1) * 128],
                                qt[qrow, mt, j * 512 + off : (j + 1) * 512],
                                start=True,
                                stop=True,
                            )
                    tot = geom[1][2] + geom[1][1]
                    for s in (0, 1):
                        pt = pt_p.tile([128, 1024], bf16, tag=f"pt{s}")
                        nc.scalar.activation(
                            pt[:, 0:tot], pss[s][:, 0:tot], exp_f, scale=0.125
                        )
                        if dg == 0:  # pair (4j, 4j+1): widths 512|384
                            nc.vector.tensor_mul(
                                pt[:, 0:896], pt[:, 0:896], mk[:, 0:896]
                            )
                        elif dg == 2:  # pair (4j+2, 4j+3): widths 256|128
                            nc.vector.tensor_mul(
                                pt[:, 0:384], pt[:, 0:384], mk[:, 896:1280]
                            )
                        pts[s][g] = (pt, geom)

                if not eg0_done:
                    emit_group(0)
                if pend_fin[0] is not None:
                    # two finalize halves bracket a filler so the PE has work
                    # while p1_ps rotates through the s0 reciprocal.  qkv
                    # fillers are safe here; out-proj fillers are NOT until
                    # f1 has written its attT columns (op_safe gating).
                    f0, f1 = pend_fin[0]
                    f0()
                    filler(0)
                    f1()
                    pend_fin[0] = None
                    op_safe[0] = True
                else:
                    filler(0)
                if ng > 1:
                    emit_group(1)

                for g in range(ng):
                    if g + 2 < ng:
                        filler(0)
                        emit_group(g + 2)
                        filler(1)
                    if g == ng - 1 and ng >= 4 and ha + 2 < HPG:
                        # pre-emit the NEXT pair's first score group before
                        # this pair's last AV group: its st buffers were
                        # freed by exp(ng-2), which has drained exactly when
                        # ScalarE would otherwise idle at the pair boundary.
                        # Feeds exp work ~3-4us earlier; skipped for ng=2
                        # chunks where the exp backlog would block the PE.
                        nctxs = [head_ctx(ha + 2), head_ctx(ha + 3)]
                        npts = {0: [None] * ng, 1: [None] * ng}
                        emit_group(0, nctxs, npts, ha + 2)
                        carry[0] = (ha + 2, nctxs, npts)
                    for u in range(2):
                        for s in (0, 1):
                            _, _, _, vsl, _, _ = ctxs[s]
                            pt, geom = pts[s][g]
                            off, w, pc = geom[u]
                            tk = 2 * g + u
                            _mm(
                                nc,
                                avs[s][0 : vsl[1] - vsl[0], off : off + w],
                                v_sb[:, tk, vsl[0] : vsl[1]],
                                pt[:, pc : pc + w],
                                start=(tk == 0),
                                stop=(tk == ntk - 1),
                            )

                filler(1)
                # sums copies go out immediately (DVE, overlaps next pair's
                # score matmuls); the rest is deferred via pend_fin.
                sums = []
                for s in (0, 1):
                    _, _, _, _, srow, _ = ctxs[s]
                    av = avs[s]
                    sums_sb = recip_p.tile([128, 512], bf16, tag=f"rc{s}")
                    nc.vector.tensor_copy(
                        sums_sb[srow : srow + 1, :], av[srow : srow + 1, :]
                    )
                    sums.append(sums_sb)

                def fin_half(s, ctxs=ctxs, avs=avs, sums=sums, ha=ha, j=j, tq=tq):
                    _, mt, _, _, srow, arow = ctxs[s]
                    bps = p1_ps.tile(
                        [128, 512], f32, tag="p1", name=f"bps_{ha}_{j}_{s}"
                    )
                    _mm(
                        nc,
                        bps[:],
                        ones_sb[srow : srow + 1, :],
                        sums[s][srow : srow + 1, :],
                        start=True,
                        stop=True,
                    )
                    bc = bcast_p.tile([128, 512], f32, tag=f"bc{s}")
                    nc.vector.reciprocal_approx_fast(bc[:], bps[:])
                    nc.vector.tensor_mul(
                        attT[arow, mt, tq], avs[s][arow, :], bc[arow, :]
                    )

                pend_fin[0] = (lambda: fin_half(0), lambda: fin_half(1))

        # ---- main schedule ----
        for u in qkv_units(0):
            u()
        for j in range(NQ):
            pend_qkv[:] = qkv_units(j + 1) if j < 3 else []
            op_safe[0] = pend_fin[0] is None
            do_chunk(j)
            # drain leftovers: next chunk's scores depend on the qkv casts;
            # out-proj leftovers are safe (this chunk consumed pend_fin).
            for u in pend_qkv:
                u()
            pend_qkv[:] = []
            for u in pend_op:
                u()
            pend_op[:] = outproj_units(j) if j < 3 else []
        f0, f1 = pend_fin[0]
        f0()
        f1()
        pend_fin[0] = None
        # last chunk's out-projection: the score pipeline is done, so the
        # st pool (2x [128,1024]) is free -- whole-t-tile blocks with the
        # o-halves alternating across the tile's two banks (same-bank
        # back-to-back accumulation runs at isolated-MM latency; alternating
        # banks pipelines at ~220ns), 2-buf pipelined so tile N+1's matmuls
        # overlap tile N's cast + DMA.
        for tl in range(4):
            tt = 12 + tl
            yps = st_ps.tile([128, 1024], f32, tag="st", name=f"yfin_{tt}")
            for k in range(4):
                for o in range(2):
                    _mm(
                        nc,
                        yps[:, o * 512 : (o + 1) * 512],
                        attT[:, k, tt * 128 : (tt + 1) * 128],
                        wo_sb[:, k, o * 512 : (o + 1) * 512],
                        start=(k == 0),
                        stop=(k == 3),
                    )
            yof = yo_p.tile([128, 1024], bf16, tag="yof", name=f"yof_{tt}")
            nc.vector.tensor_copy(yof[:], yps[:])
            nc.sync.dma_start(y[tt * 128 : (tt + 1) * 128, :], yof[:])


def _host_prep(x, w_qkv, w_out):
    def to_bf(a):
        return np.ascontiguousarray(a, dtype=np.float32).astype(ml_dtypes.bfloat16)

    xT_all = to_bf(x.transpose(0, 2, 1))
    tk_l = np.arange(128)[:, None]
    m0 = (np.arange(512)[None, :] >= tk_l).astype(np.float32)
    masks = to_bf(
        np.concatenate([m0, m0[:, :384], m0[:, :256], m0[:, :128]], axis=1)
    )

    per_group = []
    for g in range(HG):
        wq = w_qkv[g * OG : (g + 1) * OG]
        wk = w_qkv[C + g * OG : C + (g + 1) * OG]
        wvg = w_qkv[2 * C + g * OG : 2 * C + (g + 1) * OG]
        wqkT = np.concatenate([wq, wk], axis=0).T  # (C, 1024)
        wqk_r = to_bf(
            wqkT.reshape(8, 128, 8, 128).transpose(2, 1, 0, 3).reshape(8, 128, 1024)
        )
        wv_t = to_bf(wvg.T)  # (C, 512)
        wo_t = to_bf(w_out.T[g * OG : (g + 1) * OG])  # (512, C)
        per_group.append((wqk_r, wv_t, wo_t))
    vinit = np.zeros((128, 4 * PAIR_W), np.float32)
    for pr in range(4):
        vinit[:, pr * PAIR_W + 64] = 1.0  # even-head ones col
        vinit[:, pr * PAIR_W + 65] = 1.0  # odd-head ones col (block col 0)
    ones_in = to_bf(np.ones((128, 128), np.float32))
    return xT_all, masks, to_bf(vinit), ones_in, per_group


def kernel(x, w_qkv, w_out):
    x = np.asarray(x)
    w_qkv = np.asarray(w_qkv)
    w_out = np.asarray(w_out)
    xT_all, masks, vinit, ones_in, per_group = _host_prep(x, w_qkv, w_out)

    if not _NC_CACHE:
        _NC_CACHE.append(_build_nc())
    nc = _NC_CACHE[0]

    in_maps = []
    for core in range(8):
        b, g = core // 2, core % 2
        wqk_r, wv_t, wo_t = per_group[g]
        in_maps.append(
            {"xT": xT_all[b], "wqk": wqk_r, "wv": wv_t, "wo": wo_t, "masks": masks,
             "vinit": vinit, "ones_in": ones_in}
        )

    res = bass_utils.run_bass_kernel_spmd(
        nc, in_maps, core_ids=list(range(8)), trace=TRACE
    )
    LAST_RUN["res"] = res

    y = np.empty((B, T, C), np.float32)
    for b in range(B):
        y[b] = np.asarray(res.results[2 * b]["y"], np.float32) + np.asarray(
            res.results[2 * b + 1]["y"], np.float32
        )
    return y


# revision 34
# speedup vs baseline: 1.2991x; 1.0155x over previous
"""Causal self-attention (B=4, T=2048, C=1024, H=16) on 8 trn2 cores.

Sharding: batch (4-way) x head-group (2-way).  Core i handles batch i//2 and
heads [8*(i%2), 8*(i%2)+8).  Each core computes qkv projection for its head
slice, causal attention, and a partial out-projection (contraction over its
512 att columns).  Host sums the two partials per batch.

All device matmul operands are bf16 (PSUM accumulation stays fp32).  Host
pre-transposes and pre-converts so the device never transposes or converts:
  - xT       (C, T)      : x[b].T
  - wqk      (8,128,8*128): per m-tile of [wq_g; wk_g].T, k-tiles along free
  - wv       (C, 512)    : wv_g.T
  - wo       (512, C)    : w_out.T row-slice for this head group
  - masks    (128, 1280) : packed binary causal masks for the (narrowed)
                           diagonal tile pairs: [512|384|256|128] variants
  - vinit    (128, 772)  : v_sb ones/zeros init pattern
  - ones_in  (128, 128)  : all-ones lhsT for the PE sums-broadcast matmul
Layouts on chip:
  - QT/KT  [128, 4, T]   rows = head-major (hl*64+d), T on free dim
  - V      [128, 16, 772]: per t-tile, per head pair [V_e|1] + [1|0*63|V_o]
  - attT   [128, 4, T]   rows = c_local = hl*64+d  (lhsT for out-proj)

v2 structure -- fully software-pipelined phases.  The softmax exp on
ScalarE (~158us total) is the hard serial floor of the attention math, so
the kernel is organized to keep ScalarE saturated while the PE consumes
qkv-projection and out-projection work in the gaps:
  - qkv runs per t-QUARTER (512 cols): quarter 0 up front, quarter j+1
    emitted as filler units INSIDE attention chunk j.  Each unit is one
    (wave, m-tile) = 8 accumulating matmuls + one psum->sbuf cast, with
    all inputs long since DMA'd -- it can never stall the PE.
  - attention chunks processed j = 0,1,2,3; chunk j's out-projection is
    split into 8 sub-blocks (per t-tile o-half: 4 matmuls + bf16 cast +
    DMA) consumed as filler inside chunk j+1 (only after the previous
    chunk's deferred finalize has run -- they read its attT columns).
  - score matmuls for the two heads of a pair are emitted interleaved;
    s0 uses array rows 0-63, s1 rows 64-127 (tile_position auto-derived
    from base_partition), so the PE runs them CONCURRENTLY in disjoint
    row groups, halving effective score-matmul time.
  - keep-warm dummy matmuls only when no real filler is available (a PE
    idle window >~3.4us re-throttles the HAM clock gate to 1.2 GHz).
  - y is stored as bf16 (host converts + sums partials in f32).
PSUM: st 2x[128,1024] (4 banks) + av 2x[128,512] (2) + p1 2x[128,512]
(2, shared by qkv units / out-proj sub-blocks / sums-broadcast) = 8.
"""

from contextlib import ExitStack

import ml_dtypes
import numpy as np

import concourse.bass as bass
import concourse.mybir as mybir
import concourse.tile as tile
from concourse import bacc, bass_utils

B, T, C, H, HD = 4, 2048, 1024, 16, 64
HG = 2  # head groups (tensor-parallel dim)
HPG = H // HG  # 8 heads per group
OG = HPG * HD  # 512: local width of q/k/v slice
KT_C = C // 128  # 8 contraction tiles for the projections
NT = T // 128  # 16 t-tiles
NQ = T // 512  # 4 tq chunks
PAIR_W = 65 + 128  # v_sb cols per head pair: [V_e|1] + [0*63|1|V_o]
DN = 512  # keep-warm filler matmul width (~213ns at full clock)

f32 = mybir.dt.float32
bf16 = mybir.dt.bfloat16

TRACE = False  # test.py flips this for profiling runs
DEBUG = False
LAST_RUN = {}

_NC_CACHE = []


def _mm(nc, out, lhsT, rhs, **kw):
    nc.tensor.matmul(out, lhsT, rhs, **kw)


def _build_nc():
    nc = bacc.Bacc(trn_type="TRN2", target_bir_lowering=False, debug=False)
    xT = nc.dram_tensor("xT", [C, T], bf16, kind="ExternalInput").ap()
    wqk = nc.dram_tensor("wqk", [8, 128, 1024], bf16, kind="ExternalInput").ap()
    wv = nc.dram_tensor("wv", [C, OG], bf16, kind="ExternalInput").ap()
    wo = nc.dram_tensor("wo", [OG, C], bf16, kind="ExternalInput").ap()
    masks = nc.dram_tensor("masks", [128, 1280], bf16, kind="ExternalInput").ap()
    vinit = nc.dram_tensor("vinit", [128, 4 * PAIR_W], bf16, kind="ExternalInput").ap()
    ones_in = nc.dram_tensor("ones_in", [128, 128], bf16, kind="ExternalInput").ap()
    y = nc.dram_tensor("y", [T, C], bf16, kind="ExternalOutput").ap()

    with tile.TileContext(nc) as tc:
        _body(tc, nc, xT, wqk, wv, wo, masks, vinit, ones_in, y)
    nc.compile()
    return nc


def _body(tc, nc, xT, wqk, wv, wo, masks, vinit, ones_in, y):
    exp_f = mybir.ActivationFunctionType.Exp

    with (
        tc.tile_pool(name="persist", bufs=1) as persist,
        tc.tile_pool(name="mask_p", bufs=1) as mask_p,
        tc.tile_pool(name="ones_p", bufs=1) as ones_p,
        tc.tile_pool(name="wo_p", bufs=1) as wo_p,
        tc.tile_pool(name="wv_p", bufs=1) as wv_p,
        tc.tile_pool(name="xh_p", bufs=4) as xh_p,
        tc.tile_pool(name="wqk_p", bufs=1) as wqk_p,
        tc.tile_pool(name="pt_p", bufs=4) as pt_p,
        tc.tile_pool(name="recip_p", bufs=2) as recip_p,
        tc.tile_pool(name="bcast_p", bufs=2) as bcast_p,
        tc.tile_pool(name="yo_p", bufs=2) as yo_p,
        tc.tile_pool(name="st_ps", bufs=2, space="PSUM") as st_ps,
        tc.tile_pool(name="av_ps", bufs=2, space="PSUM") as av_ps,
        tc.tile_pool(name="p1_ps", bufs=2, space="PSUM") as p1_ps,
    ):
        qt = persist.tile([128, 4, T], bf16)
        kt = persist.tile([128, 4, T], bf16)
        v_sb = persist.tile([128, NT, 4 * PAIR_W], bf16)
        attT = persist.tile([128, 4, T], bf16)
        mk = mask_p.tile([128, 1280], bf16)
        ones_sb = ones_p.tile([128, 128], bf16)
        wo_sb = wo_p.tile([128, 4, C], bf16)
        wv_sb = wv_p.tile([128, KT_C, OG], bf16)

        # ---- input DMAs (ordered so the first qkv matmul's deps land first)
        wts = {}
        xh = [[None] * KT_C for _ in range(4)]  # [quarter][k]

        def load_xh(qq, k):
            t = xh_p.tile([128, 512], bf16, tag=f"xh{k}", name=f"xh_{qq}_{k}")
            xh[qq][k] = t
            nc.sync.dma_start(
                t[:], xT[k * 128 : (k + 1) * 128, qq * 512 : (qq + 1) * 512]
            )

        def load_wt(m):
            wt = wqk_p.tile([128, 1024], bf16, tag=f"wqk{m}", name=f"wt{m}")
            wts[m] = wt
            nc.sync.dma_start(wt[:], wqk[m, :, :])

        for m in range(2):
            load_wt(m)
        for k in range(KT_C):
            load_xh(0, k)
        for m in range(2, 8):
            load_wt(m)
        for k in range(KT_C):
            nc.sync.dma_start(wv_sb[:, k, :], wv[k * 128 : (k + 1) * 128, :])
        for tt in range(NT):
            nc.sync.dma_start(v_sb[:, tt, :], vinit[:])
        nc.sync.dma_start(mk[:], masks[:])
        nc.sync.dma_start(ones_sb[:], ones_in[:])
        for k in range(4):
            nc.sync.dma_start(wo_sb[:, k, :], wo[k * 128 : (k + 1) * 128, :])
        for qq in range(1, 4):
            for k in range(KT_C):
                load_xh(qq, k)

        # ---- qkv filler units: one (wave, m-tile) or (v, t-tile) each
        def qkv_units(qq):
            tq0 = qq * 512
            units = []
            for m0, dst in ((0, qt), (4, kt)):
                for mi in range(4):
                    def u(mi=mi, m0=m0, dst=dst, qq=qq, tq0=tq0):
                        ps = p1_ps.tile(
                            [128, 512], f32, tag="p1", name=f"p1_{qq}_{m0}_{mi}"
                        )
                        wt = wts[m0 + mi]
                        for k in range(KT_C):
                            _mm(
                                nc,
                                ps[:],
                                wt[:, k * 128 : (k + 1) * 128],
                                xh[qq][k][:],
                                start=(k == 0),
                                stop=(k == KT_C - 1),
                            )
                        nc.vector.tensor_copy(dst[:, mi, tq0 : tq0 + 512], ps[:])
                    units.append(u)
            for tl2 in range(4):
                def u(tl2=tl2, qq=qq):
                    tt = qq * 4 + tl2
                    ps = p1_ps.tile(
                        [128, 512], f32, tag="p1", name=f"p1v_{qq}_{tl2}"
                    )
                    for k in range(KT_C):
                        _mm(
                            nc,
                            ps[:],
                            xh[qq][k][:, tl2 * 128 : (tl2 + 1) * 128],
                            wv_sb[:, k, :],
                            start=(k == 0),
                            stop=(k == KT_C - 1),
                        )
                    src_e = ps[:].rearrange("p (h d) -> p h d", d=64)[:, 0::2, :]
                    src_o = ps[:].rearrange("p (h d) -> p h d", d=64)[:, 1::2, :]
                    dstv = v_sb[:, tt, :].rearrange("p (q w) -> p q w", w=PAIR_W)
                    nc.vector.tensor_copy(dstv[:, :, 0:64], src_e)
                    nc.vector.tensor_copy(dstv[:, :, 129:193], src_o)
                units.append(u)
            return units

        # ---- out-projection sub-blocks (filler units for the next chunk)
        def outproj_sub(tt, o):
            yps = p1_ps.tile([128, 512], f32, tag="p1", name=f"yp_{tt}_{o}")
            for k in range(4):
                _mm(
                    nc,
                    yps[:],
                    attT[:, k, tt * 128 : (tt + 1) * 128],
                    wo_sb[:, k, o * 512 : (o + 1) * 512],
                    start=(k == 0),
                    stop=(k == 3),
                )
            yo = yo_p.tile([128, 512], bf16, tag="yo", name=f"yo_{tt}_{o}")
            nc.vector.tensor_copy(yo[:], yps[:])
            nc.sync.dma_start(
                y[tt * 128 : (tt + 1) * 128, o * 512 : (o + 1) * 512], yo[:]
            )

        def outproj_units(j):
            return [
                (lambda tt, oo: (lambda: outproj_sub(tt, oo)))(4 * j + tl, o)
                for tl in range(4)
                for o in range(2)
            ]

        # ---- attention ----
        def head_ctx(hl):
            p0 = (hl % 2) * 64
            mt = hl // 2
            qrow = slice(p0, p0 + 64)
            vb0 = (hl // 2) * PAIR_W
            if hl % 2 == 0:
                vsl = (vb0, vb0 + 65)  # [V|1] -> rows 0..64
                srow, arow = 64, slice(0, 64)
            else:
                vsl = (vb0 + 65, vb0 + 193)  # [1|0*63|V] -> row 0 sums
                srow, arow = 0, slice(64, 128)
            return p0, mt, qrow, vsl, srow, arow

        pend_fin = [None]  # deferred finalize of the previous head pair
        pend_qkv = []  # next quarter's qkv units (safe any time)
        pend_op = []  # prev chunk's out-proj units (safe after pend_fin)
        op_safe = [False]

        def do_chunk(j):
            ntk = 4 * j + 4
            ng = ntk // 2
            tq = slice(j * 512, (j + 1) * 512)
            for ha in range(0, HPG, 2):
                ctxs = [head_ctx(ha), head_ctx(ha + 1)]
                pts = {0: [None] * ng, 1: [None] * ng}
                avs = [
                    av_ps.tile([128, 512], f32, tag="av", name=f"av{s}_{ha}_{j}")
                    for s in (0, 1)
                ]

                def dummy(s):
                    # keep-warm filler matmul into the dead rows of the av
                    # accumulator (the complement of each head's rows is
                    # never read).  A stalled PE re-throttles the HAM clock
                    # gate to 1.2 GHz, doubling every matmul.
                    rows = slice(96, 128) if s == 0 else slice(32, 64)
                    _mm(
                        nc,
                        avs[s][rows, 0:DN],
                        mk[:, 64 : 64 + (rows.stop - rows.start)],
                        mk[:, 0:DN],
                        start=True,
                        stop=True,
                        skip_group_check=True,
                        tile_position=(0, rows.start),
                    )

                def filler(s):
                    if pend_qkv:
                        pend_qkv.pop(0)()
                    elif op_safe[0] and pend_op:
                        pend_op.pop(0)()
                    else:
                        dummy(s)

                def emit_group(g):
                    geom = []  # per u: (tq_off, width, pt_col)
                    pcol = 0
                    for u in range(2):
                        tk = 2 * g + u
                        v = tk - 4 * j
                        off = 128 * v if v > 0 else 0
                        w = 512 - off
                        geom.append((off, w, pcol))
                        pcol = 512 if u == 0 and w == 512 else pcol + w
                    dg = 2 * g - 4 * j
                    pss = [
                        st_ps.tile(
                            [128, 1024], f32, tag="st", name=f"st{s}_{ha}_{j}_{g}"
                        )
                        for s in (0, 1)
                    ]
                    # two heads' score matmuls interleaved: s0 in array rows
                    # 0-63, s1 in rows 64-127 -> concurrent row groups.
                    for u in range(2):
                        off, w, pc = geom[u]
                        tk = 2 * g + u
                        for s in (0, 1):
                            _, mt, qrow, _, _, _ = ctxs[s]
                            _mm(
                                nc,
                                pss[s][:, pc : pc + w],
                                kt[qrow, mt, tk * 128 : (tk + 1) * 128],
                                qt[qrow, mt, j * 512 + off : (j + 1) * 512],
                                start=True,
                                stop=True,
                            )
                    tot = geom[1][2] + geom[1][1]
                    for s in (0, 1):
                        pt = pt_p.tile([128, 1024], bf16, tag=f"pt{s}")
                        nc.scalar.activation(
                            pt[:, 0:tot], pss[s][:, 0:tot], exp_f, scale=0.125
                        )
                        if dg == 0:  # pair (4j, 4j+1): widths 512|384
                            nc.vector.tensor_mul(
                                pt[:, 0:896], pt[:, 0:896], mk[:, 0:896]
                            )
                        elif dg == 2:  # pair (4j+2, 4j+3): widths 256|128
                            nc.vector.tensor_mul(
                                pt[:, 0:384], pt[:, 0:384], mk[:, 896:1280]
                            )
                        pts[s][g] = (pt, geom)

                emit_group(0)
                if pend_fin[0] is not None:
                    # two finalize halves bracket a filler so the PE has work
                    # while p1_ps rotates through the s0 reciprocal.  qkv
                    # fillers are safe here; out-proj fillers are NOT until
                    # f1 has written its attT columns (op_safe gating).
                    f0, f1 = pend_fin[0]
                    f0()
                    filler(0)
                    f1()
                    pend_fin[0] = None
                    op_safe[0] = True
                else:
                    filler(0)
                if ng > 1:
                    emit_group(1)

                for g in range(ng):
                    if g + 2 < ng:
                        filler(0)
                        emit_group(g + 2)
                        filler(1)
                    for u in range(2):
                        for s in (0, 1):
                            _, _, _, vsl, _, _ = ctxs[s]
                            pt, geom = pts[s][g]
                            off, w, pc = geom[u]
                            tk = 2 * g + u
                            _mm(
                                nc,
                                avs[s][0 : vsl[1] - vsl[0], off : off + w],
                                v_sb[:, tk, vsl[0] : vsl[1]],
                                pt[:, pc : pc + w],
                                start=(tk == 0),
                                stop=(tk == ntk - 1),
                            )

                filler(1)
                # sums copies go out immediately (DVE, overlaps next pair's
                # score matmuls); the rest is deferred via pend_fin.
                sums = []
                for s in (0, 1):
                    _, _, _, _, srow, _ = ctxs[s]
                    av = avs[s]
                    sums_sb = recip_p.tile([128, 512], bf16, tag=f"rc{s}")
                    nc.vector.tensor_copy(
                        sums_sb[srow : srow + 1, :], av[srow : srow + 1, :]
                    )
                    sums.append(sums_sb)

                def fin_half(s, ctxs=ctxs, avs=avs, sums=sums, ha=ha, j=j, tq=tq):
                    _, mt, _, _, srow, arow = ctxs[s]
                    bps = p1_ps.tile(
                        [128, 512], f32, tag="p1", name=f"bps_{ha}_{j}_{s}"
                    )
                    _mm(
                        nc,
                        bps[:],
                        ones_sb[srow : srow + 1, :],
                        sums[s][srow : srow + 1, :],
                        start=True,
                        stop=True,
                    )
                    bc = bcast_p.tile([128, 512], f32, tag=f"bc{s}")
                    nc.vector.reciprocal_approx_fast(bc[:], bps[:])
                    nc.vector.tensor_mul(
                        attT[arow, mt, tq], avs[s][arow, :], bc[arow, :]
                    )

                pend_fin[0] = (lambda: fin_half(0), lambda: fin_half(1))

        # ---- main schedule ----
        for u in qkv_units(0):
            u()
        for j in range(NQ):
            pend_qkv[:] = qkv_units(j + 1) if j < 3 else []
            op_safe[0] = pend_fin[0] is None
            do_chunk(j)
            # drain leftovers: next chunk's scores depend on the qkv casts;
            # out-proj leftovers are safe (this chunk consumed pend_fin).
            for u in pend_qkv:
                u()
            pend_qkv[:] = []
            for u in pend_op:
                u()
            pend_op[:] = outproj_units(j) if j < 3 else []
        f0, f1 = pend_fin[0]
        f0()
        f1()
        pend_fin[0] = None
        # last chunk's out-projection: the score pipeline is done, so the
        # st pool (2x [128,1024]) is free -- whole-t-tile blocks with the
        # o-halves alternating across the tile's two banks (same-bank
        # back-to-back accumulation runs at isolated-MM latency; alternating
        # banks pipelines at ~220ns), 2-buf pipelined so tile N+1's matmuls
        # overlap tile N's cast + DMA.
        for tl in range(4):
            tt = 12 + tl
            yps = st_ps.tile([128, 1024], f32, tag="st", name=f"yfin_{tt}")
            for k in range(4):
                for o in range(2):
                    _mm(
                        nc,
                        yps[:, o * 512 : (o + 1) * 512],
                        attT[:, k, tt * 128 : (tt + 1) * 128],
                        wo_sb[:, k, o * 512 : (o + 1) * 512],
                        start=(k == 0),
                        stop=(k == 3),
                    )
            yof = yo_p.tile([128, 1024], bf16, tag="yof", name=f"yof_{tt}")
            nc.vector.tensor_copy(yof[:], yps[:])
            nc.sync.dma_start(y[tt * 128 : (tt + 1) * 128, :], yof[:])


def _host_prep(x, w_qkv, w_out):
    def to_bf(a):
        return np.ascontiguousarray(a, dtype=np.float32).astype(ml_dtypes.bfloat16)

    xT_all = to_bf(x.transpose(0, 2, 1))
    tk_l = np.arange(128)[:, None]
    m0 = (np.arange(512)[None, :] >= tk_l).astype(np.float32)
    masks = to_bf(
        np.concatenate([m0, m0[:, :384], m0[:, :256], m0[:, :128]], axis=1)
    )

    per_group = []
    for g in range(HG):
        wq = w_qkv[g * OG : (g + 1) * OG]
        wk = w_qkv[C + g * OG : C + (g + 1) * OG]
        wvg = w_qkv[2 * C + g * OG : 2 * C + (g + 1) * OG]
        wqkT = np.concatenate([wq, wk], axis=0).T  # (C, 1024)
        wqk_r = to_bf(
            wqkT.reshape(8, 128, 8, 128).transpose(2, 1, 0, 3).reshape(8, 128, 1024)
        )
        wv_t = to_bf(wvg.T)  # (C, 512)
        wo_t = to_bf(w_out.T[g * OG : (g + 1) * OG])  # (512, C)
        per_group.append((wqk_r, wv_t, wo_t))
    vinit = np.zeros((128, 4 * PAIR_W), np.float32)
    for pr in range(4):
        vinit[:, pr * PAIR_W + 64] = 1.0  # even-head ones col
        vinit[:, pr * PAIR_W + 65] = 1.0  # odd-head ones col (block col 0)
    ones_in = to_bf(np.ones((128, 128), np.float32))
    return xT_all, masks, to_bf(vinit), ones_in, per_group


def kernel(x, w_qkv, w_out):
    x = np.asarray(x)
    w_qkv = np.asarray(w_qkv)
    w_out = np.asarray(w_out)
    xT_all, masks, vinit, ones_in, per_group = _host_prep(x, w_qkv, w_out)

    if not _NC_CACHE:
        _NC_CACHE.append(_build_nc())
    nc = _NC_CACHE[0]

    in_maps = []
    for core in range(8):
        b, g = core // 2, core % 2
        wqk_r, wv_t, wo_t = per_group[g]
        in_maps.append(
            {"xT": xT_all[b], "wqk": wqk_r, "wv": wv_t, "wo": wo_t, "masks": masks,
             "vinit": vinit, "ones_in": ones_in}
        )

    res = bass_utils.run_bass_kernel_spmd(
        nc, in_maps, core_ids=list(range(8)), trace=TRACE
    )
    LAST_RUN["res"] = res

    y = np.empty((B, T, C), np.float32)
    for b in range(B):
        y[b] = np.asarray(res.results[2 * b]["y"], np.float32) + np.asarray(
            res.results[2 * b + 1]["y"], np.float32
        )
    return y
